# revision 10
# baseline (speedup 1.0000x reference)
"""Trainium2 Bass kernel for nn_COMSAGPool_gumble (GNN message passing).

Math (reference):
    prop(x) = nd * segsum_dst((ns * x)[src]),  ns/nd = rsqrt(clipped degrees)
    out1 = prop(F) @ W1 + b1 ; out2 = prop(out1 @ W2) + b2
    mask = onehot(argmax(out2 + gumbel)); p = softmax(out2)
    fdis = F * tanh(p0) * mask0 ; fcom = F * tanh(p1) * mask1

Rewrite (prop is linear, commutes with right-mult):
    out2 = prop(prop(F @ (W1@W2))) + q * (b1@W2) + b2,  q = prop(ones)
so both sparse hops run on 2 feature columns.

Distribution: nodes sharded over 8 cores; node->slot assignment is
degree-sorted so each slot column has a uniform per-node edge capacity
(fixed-stride segment sums, no boundary extraction). Per-edge values are
fetched with dma_gather (256B items, 4 node-values packed per item,
sub-position resolved with host-built masks). Hop outputs are exchanged
with AllGather; the host reassembles outputs from slot order.
"""
import os
import numpy as np

import concourse.bacc as bacc
import concourse.mybir as mybir
import concourse.tile as tile
from concourse import bass_utils, library_config
from concourse.tile_rust import add_dep_helper

N = 100000
E = 1600000
D = 128
NCORES = 8
TC = 98                      # slot columns; 8*128*98 = 100352 slots
SLOTS = 128 * TC             # 12544 per core
GRID = NCORES * 128          # 1024 grid rows (core, partition)
PADQ = 4
NITEMS = (NCORES * SLOTS) // 4   # 25088 packed 256B items
EW = 64                      # floats per item
NIDX_CALL = 1024             # dma_gather per-call cap
CCB = 8                      # calls per staging buffer (8*1024/128 = 64 cols)
NQ = 4
F32 = mybir.dt.float32

_CACHE = {}
LAST_RESULTS = None


def _prep(src, dst, feature, gumbel, W1, b1, W2, b2):
    src = np.asarray(src).astype(np.int64)
    dst = np.asarray(dst).astype(np.int64)

    deg_out = np.bincount(src, minlength=N)
    deg_in = np.bincount(dst, minlength=N)
    ns = (1.0 / np.sqrt(np.maximum(deg_out, 1).astype(np.float32))).astype(np.float32)
    nd = (1.0 / np.sqrt(np.maximum(deg_in, 1).astype(np.float32))).astype(np.float32)

    # degree-sorted node -> (core, partition, column) assignment
    degK = ((deg_in + PADQ - 1) // PADQ) * PADQ
    order_nodes = np.argsort(-degK, kind="stable")
    rank = np.empty(N, np.int64)
    rank[order_nodes] = np.arange(N)
    t_n = rank // GRID
    g_n = rank % GRID
    c_n = g_n // 128
    p_n = g_n % 128
    # per-column edge capacity (uniform across the whole grid)
    ncols_used = (N + GRID - 1) // GRID
    Kt = np.zeros(TC, np.int64)
    first = np.minimum(np.arange(ncols_used) * GRID, N - 1)
    Kt[:ncols_used] = degK[order_nodes[first]]
    off_t = np.zeros(TC + 1, np.int64)
    np.cumsum(Kt, out=off_t[1:])
    L = int(off_t[-1])
    Lpad = ((L + CCB * 8 - 1) // (CCB * 8)) * (CCB * 8)
    ncalls = (Lpad * 128) // NIDX_CALL

    # cc row u, packed item, sub-position
    u_n = c_n * SLOTS + p_n * TC + t_n
    item_n = (u_n // 4).astype(np.int16)
    sub_n = (u_n % 4).astype(np.int8)

    # edge -> (core, partition, column, slot col j)
    cd, pd, td = c_n[dst], p_n[dst], t_n[dst]
    gkey = ((cd * 128 + pd) * TC + td).astype(np.int64)
    eorder = np.argsort(gkey, kind="stable")
    gkey_s = gkey[eorder]
    cnt = np.bincount(gkey, minlength=GRID * TC)
    gstart = np.zeros(GRID * TC + 1, np.int64)
    np.cumsum(cnt, out=gstart[1:])
    erank = np.arange(E, dtype=np.int64) - gstart[gkey_s]
    j_s = off_t[gkey_s % TC] + erank
    assert int(j_s.max()) < L

    # per-core gather item grids + sub-position masks
    items = np.zeros((NCORES, 128, Lpad), np.int16)
    masks = np.zeros((NCORES, 4, 128, Lpad), np.float32)
    cds, pds = cd[eorder], pd[eorder]
    src_s = src[eorder]
    items[cds, pds, j_s] = item_n[src_s]
    m_sel = sub_n[src_s]
    for k in range(4):
        sel = m_sel == k
        masks[cds[sel], k, pds[sel], j_s[sel]] = 1.0

    # wrapped int16 idx tiles: call q covers seq i = (col-j local)*128 + p
    seq = items.transpose(0, 2, 1).reshape(NCORES, Lpad * 128)
    seq = seq.reshape(NCORES, ncalls, NIDX_CALL // 16, 16)
    idxw = np.zeros((NCORES, 128, ncalls * (NIDX_CALL // 16)), np.int16)
    for p in range(128):
        idxw[:, p, :] = seq[:, :, :, p % 16].reshape(NCORES, -1)

    # contiguous column ranges with equal K > 0
    ranges = []
    t0 = 0
    while t0 < TC:
        if Kt[t0] == 0:
            t0 += 1
            continue
        t1 = t0
        while t1 < TC and Kt[t1] == Kt[t0]:
            t1 += 1
        ranges.append((int(t0), int(t1), int(Kt[t0]), int(off_t[t0])))
        t0 = t1

    def plane(val):
        a = np.zeros((NCORES, 128, TC), np.float32)
        a[c_n, p_n, t_n] = val
        return a

    def plane2(val):
        a = plane(val)
        return np.repeat(a.reshape(NCORES, 128, TC, 1), 2, axis=3).reshape(
            NCORES, 128, 2 * TC
        )

    W1 = np.asarray(W1, np.float32)
    W2 = np.asarray(W2, np.float32)
    b1 = np.asarray(b1, np.float32)
    b2 = np.asarray(b2, np.float32)
    W12 = (W1 @ W2).astype(np.float32)
    b1W2 = (b1 @ W2).astype(np.float32)
    q_vec = (np.bincount(dst, weights=ns[src].astype(np.float64), minlength=N)
             .astype(np.float32) * nd)
    qb2 = np.zeros((NCORES, 128, 2 * TC), np.float32)
    qp = plane(q_vec)
    for ch in range(2):
        qb2[:, :, ch::2] = qp * b1W2[ch] + b2[ch]
    gum = np.asarray(gumbel, np.float32)
    gd = plane(gum[:, 0] - gum[:, 1])

    slot2node = np.full((NCORES, SLOTS), -1, np.int64)
    slot2node[c_n, p_n * TC + t_n] = np.arange(N)
    feature = np.asarray(feature, np.float32)
    fs, fts = [], []
    for c in range(NCORES):
        s2n = slot2node[c]
        fc = np.zeros((SLOTS, D), np.float32)
        valid = s2n >= 0
        fc[valid] = feature[s2n[valid]]
        fs.append(fc)
        # FT column t*128+p must hold slot p*TC+t (the P' matmul writes
        # column-tile t to psum partitions p)
        j = np.arange(SLOTS)
        perm = (j % 128) * TC + (j // 128)
        fts.append(np.ascontiguousarray(fc[perm].T))

    return dict(
        L=L, Lpad=Lpad, ncalls=ncalls, ranges=ranges,
        idxw=idxw, masks=masks, fs=fs, fts=fts, W12=W12,
        ns2=plane2(ns), v1=plane2(nd * ns), nd2=plane2(nd), qb2=qb2, gd=gd,
        slot2node=slot2node,
    )


def _build(Lpad, ncalls, ranges, debug=False):
    nc = bacc.Bacc("TRN2", target_bir_lowering=False, debug=False,
                   num_devices=NCORES, num_swdge_queues=NQ)
    AOT = mybir.AluOpType
    groups = [list(range(NCORES))]

    def di(name, shape, dt=F32):
        return nc.dram_tensor(name, shape, dt, kind="ExternalInput").ap()

    FT = di("ft", [128, SLOTS])
    W12 = di("w12", [128, 2])
    IDX = di("idx", [128, ncalls * (NIDX_CALL // 16)], mybir.dt.int16)
    MK = di("mk", [4, 128, Lpad])
    NS2 = di("ns2", [128, 2 * TC])
    V1 = di("v1", [128, 2 * TC])
    ND2 = di("nd2", [128, 2 * TC])
    QB2 = di("qb2", [128, 2 * TC])
    GD = di("gd", [128, TC])
    F = di("f", [SLOTS, D])

    FDIS = nc.dram_tensor("fdis", [SLOTS, D], F32, kind="ExternalOutput").ap()
    FCOM = nc.dram_tensor("fcom", [SLOTS, D], F32, kind="ExternalOutput").ap()
    M0 = nc.dram_tensor("m0", [SLOTS], F32, kind="ExternalOutput").ap()
    M1 = nc.dram_tensor("m1", [SLOTS], F32, kind="ExternalOutput").ap()

    ccPin = nc.dram_tensor("ccpin", [SLOTS, 2], F32, kind="Internal").ap()
    ccPout = nc.dram_tensor("ccpout", [NCORES * SLOTS, 2], F32,
                            kind="Internal", addr_space="Shared").ap()
    ccQin = nc.dram_tensor("ccqin", [SLOTS, 2], F32, kind="Internal").ap()
    ccQout = nc.dram_tensor("ccqout", [NCORES * SLOTS, 2], F32,
                            kind="Internal", addr_space="Shared").ap()
    padP = nc.dram_tensor("padp", [NITEMS, EW], F32, kind="Internal").ap()
    padQ = nc.dram_tensor("padq", [NITEMS, EW], F32, kind="Internal").ap()
    barin = nc.dram_tensor("barin", [128, 2], F32, kind="Internal").ap()
    barout = nc.dram_tensor("barout", [128, 2], F32, kind="Internal").ap()
    dbg = {}
    if debug:
        for nm, shp in [("dbg_v0", [128, Lpad]), ("dbg_v1", [128, Lpad]),
                        ("dbg_cmp", [128, NCORES * SLOTS * 2 // 128]),
                        ("dbg_s0", [128, TC]), ("dbg_s1", [128, TC])]:
            dbg[nm] = nc.dram_tensor(nm, shp, F32, kind="ExternalOutput").ap()

    with tile.TileContext(nc) as tc:
        with (
            tc.tile_pool(name="const", bufs=1) as kpool,
            tc.tile_pool(name="gbuf", bufs=2) as gpool,
            tc.tile_pool(name="vals", bufs=1) as vpool,
            tc.tile_pool(name="nodes", bufs=1) as npool,
            tc.tile_pool(name="fio", bufs=2) as fpool,
            tc.tile_pool(name="psum", bufs=1, space="PSUM") as ppool,
        ):
            nc.gpsimd.load_library(library_config.mlp)
            idx_sb = kpool.tile([128, ncalls * (NIDX_CALL // 16)], mybir.dt.int16)
            nc.sync.dma_start(idx_sb[:], IDX)
            mk_sb = []
            for k in range(4):
                t = kpool.tile([128, Lpad], F32, tag=f"mk{k}")
                nc.sync.dma_start(t[:], MK[k])
                mk_sb.append(t)
            w12_sb = kpool.tile([128, 2], F32)
            nc.sync.dma_start(w12_sb[:], W12)
            ns2_sb = kpool.tile([128, 2 * TC], F32)
            nc.sync.dma_start(ns2_sb[:], NS2)
            v1_sb = kpool.tile([128, 2 * TC], F32)
            nc.sync.dma_start(v1_sb[:], V1)
            nd2_sb = kpool.tile([128, 2 * TC], F32)
            nc.sync.dma_start(nd2_sb[:], ND2)
            qb2_sb = kpool.tile([128, 2 * TC], F32)
            nc.sync.dma_start(qb2_sb[:], QB2)
            gd_sb = kpool.tile([128, TC], F32)
            nc.sync.dma_start(gd_sb[:], GD)

            # ---- P' = ns * (F @ W12) via PE (FT is host-transposed F) ----
            with tc.tile_pool(name="ftw", bufs=2) as ftpool:
                p_psum = ppool.tile([128, 2 * TC], F32)
                FTCH = 8
                for t0c in range(0, TC, FTCH):
                    nbt = min(FTCH, TC - t0c)
                    ft_sb = ftpool.tile([128, FTCH * 128], F32, tag="ftc")
                    nc.sync.dma_start(
                        ft_sb[:, 0 : nbt * 128],
                        FT[:, t0c * 128 : (t0c + nbt) * 128],
                    )
                    for j in range(nbt):
                        t = t0c + j
                        nc.tensor.matmul(
                            p_psum[:, 2 * t : 2 * t + 2],
                            lhsT=ft_sb[:, j * 128 : (j + 1) * 128],
                            rhs=w12_sb[:],
                            start=True, stop=True,
                        )
                pp_sb = npool.tile([128, 2 * TC], F32, tag="pp")
                nc.vector.tensor_tensor(pp_sb[:], p_psum[:], ns2_sb[:], AOT.mult)

            def exchange(plane_sb, cc_in, cc_out, pad):
                """compact write -> AllGather -> expand into packed items."""
                nc.sync.dma_start(
                    cc_in.rearrange("(p t) c -> p (t c)", p=128), plane_sb[:]
                )
                nc.gpsimd.collective_compute(
                    "AllGather", AOT.bypass, replica_groups=groups,
                    ins=[cc_in], outs=[cc_out],
                )
                # barrier collective: ring-ordered execution means its
                # completion implies the AllGather's remote data has landed
                bar = nc.gpsimd.collective_compute(
                    "AllReduce", AOT.add, replica_groups=groups,
                    ins=[barin], outs=[barout],
                )
                cmp_sb = vpool.tile([128, NCORES * SLOTS * 2 // 128], F32,
                                    tag="cmp")
                rd = nc.sync.dma_start(
                    cmp_sb[:], cc_out.rearrange("(p a) c -> p (a c)", p=128)
                )
                add_dep_helper(rd.ins, bar.ins, reason="cc data barrier")
                if debug and cc_in is ccPin:
                    nc.sync.dma_start(dbg["dbg_cmp"], cmp_sb[:])
                nc.sync.dma_start(
                    pad.rearrange("(p i) w -> p i w", p=128)[:, :, 0:8],
                    cmp_sb[:].rearrange("p (i e) -> p i e", e=8),
                )

            exchange(pp_sb, ccPin, ccPout, padP)

            # ---- one hop: gather packed items, mask-select, fixed-K reduce --
            def hop(pad_src, tag):
                val0 = vpool.tile([128, Lpad], F32, tag="va")
                val1 = vpool.tile([128, Lpad], F32, tag="vb")
                bcols = CCB * NIDX_CALL // 128      # columns per staging buf
                nblk = Lpad // bcols
                for b in range(nblk):
                    gt = gpool.tile([128, bcols, EW], F32, tag="g")
                    for qq in range(CCB):
                        q = b * CCB + qq
                        nc.gpsimd.dma_gather(
                            out_ap=gt[:, qq * 8 : (qq + 1) * 8, :],
                            in_ap=pad_src,
                            idxs_ap=idx_sb[
                                :,
                                q * (NIDX_CALL // 16) : (q + 1) * (NIDX_CALL // 16),
                            ],
                            num_idxs=NIDX_CALL,
                            num_idxs_reg=NIDX_CALL,
                            elem_size=EW,
                            queue_num=q % NQ,
                        )
                    c0 = b * bcols
                    for cch, vt in ((0, val0), (1, val1)):
                        tmp = gpool.tile([128, bcols], F32, tag=f"t{cch}")
                        tmp2 = gpool.tile([128, bcols], F32, tag=f"u{cch}")
                        nc.vector.tensor_tensor(
                            tmp[:], gt[:, :, 0 + cch],
                            mk_sb[0][:, c0 : c0 + bcols], AOT.mult)
                        nc.vector.tensor_tensor(
                            tmp2[:], gt[:, :, 2 + cch],
                            mk_sb[1][:, c0 : c0 + bcols], AOT.mult)
                        nc.vector.tensor_tensor(tmp[:], tmp[:], tmp2[:], AOT.add)
                        nc.vector.tensor_tensor(
                            tmp2[:], gt[:, :, 4 + cch],
                            mk_sb[2][:, c0 : c0 + bcols], AOT.mult)
                        nc.vector.tensor_tensor(tmp[:], tmp[:], tmp2[:], AOT.add)
                        nc.vector.tensor_tensor(
                            tmp2[:], gt[:, :, 6 + cch],
                            mk_sb[3][:, c0 : c0 + bcols], AOT.mult)
                        nc.vector.tensor_tensor(
                            vt[:, c0 : c0 + bcols], tmp[:], tmp2[:], AOT.add)
                if debug and tag == "a":
                    nc.sync.dma_start(dbg["dbg_v0"], val0[:])
                    nc.sync.dma_start(dbg["dbg_v1"], val1[:])
                seg0 = npool.tile([128, TC], F32, tag=f"sa{tag}")
                seg1 = npool.tile([128, TC], F32, tag=f"sb{tag}")
                nc.vector.memset(seg0[:], 0.0)
                nc.vector.memset(seg1[:], 0.0)
                for (t0, t1, K, off) in ranges:
                    ntr = t1 - t0
                    for vt, st in ((val0, seg0), (val1, seg1)):
                        nc.vector.tensor_reduce(
                            st[:, t0:t1],
                            vt[:, off : off + ntr * K].rearrange(
                                "p (a k) -> p a k", k=K
                            ),
                            axis=mybir.AxisListType.X,
                            op=AOT.add,
                        )
                return seg0, seg1

            s10, s11 = hop(padP, "a")
            if debug:
                nc.sync.dma_start(dbg["dbg_s0"], s10[:])
                nc.sync.dma_start(dbg["dbg_s1"], s11[:])
            qp_sb = npool.tile([128, 2 * TC], F32, tag="qp")
            qp3 = qp_sb[:].rearrange("p (t c) -> p t c", c=2)
            v13 = v1_sb[:].rearrange("p (t c) -> p t c", c=2)
            nc.vector.tensor_tensor(qp3[:, :, 0], s10[:], v13[:, :, 0], AOT.mult)
            nc.vector.tensor_tensor(qp3[:, :, 1], s11[:], v13[:, :, 1], AOT.mult)
            exchange(qp_sb, ccQin, ccQout, padQ)
            s20, s21 = hop(padQ, "b")

            # ---- head ----
            nd3 = nd2_sb[:].rearrange("p (t c) -> p t c", c=2)
            qb3 = qb2_sb[:].rearrange("p (t c) -> p t c", c=2)
            o20a = npool.tile([128, TC], F32, tag="o20a")
            o21a = npool.tile([128, TC], F32, tag="o21a")
            nc.vector.tensor_tensor(o20a[:], s20[:], nd3[:, :, 0], AOT.mult)
            nc.vector.tensor_tensor(o21a[:], s21[:], nd3[:, :, 1], AOT.mult)
            o20 = npool.tile([128, TC], F32, tag="o20")
            o21 = npool.tile([128, TC], F32, tag="o21")
            nc.vector.tensor_tensor(o20[:], o20a[:], qb3[:, :, 0], AOT.add)
            nc.vector.tensor_tensor(o21[:], o21a[:], qb3[:, :, 1], AOT.add)
            d01 = npool.tile([128, TC], F32, tag="d01")
            nc.vector.tensor_tensor(d01[:], o20[:], o21[:], AOT.subtract)
            m = npool.tile([128, TC], F32, tag="m")
            nc.vector.tensor_tensor(m[:], d01[:], gd_sb[:], AOT.add)
            mask0 = npool.tile([128, TC], F32, tag="mask0")
            mask1 = npool.tile([128, TC], F32, tag="mask1")
            nc.vector.tensor_scalar(mask0[:], m[:], 0.0, None, AOT.is_ge)
            nc.vector.tensor_scalar(mask1[:], m[:], 0.0, None, AOT.is_lt)
            p0 = npool.tile([128, TC], F32, tag="p0")
            p1 = npool.tile([128, TC], F32, tag="p1")
            nc.scalar.activation(p0[:], d01[:], mybir.ActivationFunctionType.Sigmoid)
            nc.scalar.activation(p1[:], d01[:], mybir.ActivationFunctionType.Sigmoid,
                                 scale=-1.0)
            t0_ = npool.tile([128, TC], F32, tag="t0")
            t1_ = npool.tile([128, TC], F32, tag="t1")
            nc.scalar.activation(t0_[:], p0[:], mybir.ActivationFunctionType.Tanh)
            nc.scalar.activation(t1_[:], p1[:], mybir.ActivationFunctionType.Tanh)
            s0 = npool.tile([128, TC], F32, tag="s0")
            s1 = npool.tile([128, TC], F32, tag="s1")
            nc.vector.tensor_tensor(s0[:], t0_[:], mask0[:], AOT.mult)
            nc.vector.tensor_tensor(s1[:], t1_[:], mask1[:], AOT.mult)

            nc.sync.dma_start(M0.rearrange("(p t) -> p t", p=128), mask0[:])
            nc.sync.dma_start(M1.rearrange("(p t) -> p t", p=128), mask1[:])

            # ---- final: fdis/fcom = F * s0/s1, batched over t ----
            NB = 4
            for tb in range(0, TC, NB):
                nb = min(NB, TC - tb)
                fin = fpool.tile([128, NB, D], F32, tag="fin")
                nc.sync.dma_start(
                    fin[:, 0:nb, :],
                    F.rearrange("(p t) d -> p t d", p=128)[:, tb : tb + nb, :],
                )
                fd = fpool.tile([128, NB, D], F32, tag="fd")
                fc = fpool.tile([128, NB, D], F32, tag="fc")
                for j in range(nb):
                    t = tb + j
                    nc.vector.tensor_scalar(
                        fd[:, j, :], fin[:, j, :], s0[:, t : t + 1], None, AOT.mult)
                    nc.vector.tensor_scalar(
                        fc[:, j, :], fin[:, j, :], s1[:, t : t + 1], None, AOT.mult)
                nc.sync.dma_start(
                    FDIS.rearrange("(p t) d -> p t d", p=128)[:, tb : tb + nb, :],
                    fd[:, 0:nb, :])
                nc.sync.dma_start(
                    FCOM.rearrange("(p t) d -> p t d", p=128)[:, tb : tb + nb, :],
                    fc[:, 0:nb, :])

    nc.compile()
    return nc


def kernel(feature, src, dst, gumbel, label, W1, b1, W2, b2):
    global LAST_RESULTS
    pre = _prep(src, dst, feature, gumbel, W1, b1, W2, b2)
    key = (pre["Lpad"], pre["ncalls"], tuple(pre["ranges"]))
    debug = bool(os.environ.get("KERNEL_DEBUG"))
    key = key + (debug,)
    if key not in _CACHE:
        _CACHE[key] = _build(pre["Lpad"], pre["ncalls"], pre["ranges"],
                             debug=debug)
    nc = _CACHE[key]

    in_maps = []
    for c in range(NCORES):
        in_maps.append({
            "ft": pre["fts"][c],
            "w12": np.ascontiguousarray(pre["W12"]),
            "idx": np.ascontiguousarray(pre["idxw"][c]),
            "mk": np.ascontiguousarray(pre["masks"][c]),
            "ns2": np.ascontiguousarray(pre["ns2"][c]),
            "v1": np.ascontiguousarray(pre["v1"][c]),
            "nd2": np.ascontiguousarray(pre["nd2"][c]),
            "qb2": np.ascontiguousarray(pre["qb2"][c]),
            "gd": np.ascontiguousarray(pre["gd"][c]),
            "f": pre["fs"][c],
        })

    trace = os.environ.get("KERNEL_TRACE", "") not in ("", "0")
    try:
        res = bass_utils.run_bass_kernel_spmd(
            nc, in_maps, core_ids=list(range(NCORES)), trace=trace
        )
    except ModuleNotFoundError:
        os.environ["BASS_NEVER_TRACE"] = "1"
        res = bass_utils.run_bass_kernel_spmd(
            nc, in_maps, core_ids=list(range(NCORES))
        )
    LAST_RESULTS = res

    fdis = np.zeros((N, D), np.float32)
    fcom = np.zeros((N, D), np.float32)
    m0 = np.zeros(N, np.float32)
    m1 = np.zeros(N, np.float32)
    for c in range(NCORES):
        s2n = pre["slot2node"][c]
        valid = s2n >= 0
        nodes = s2n[valid]
        fdis[nodes] = res.results[c]["fdis"][valid]
        fcom[nodes] = res.results[c]["fcom"][valid]
        m0[nodes] = res.results[c]["m0"][valid]
        m1[nodes] = res.results[c]["m1"][valid]
    return fdis, fcom, m0, m1


# revision 11
# speedup vs baseline: 1.0060x; 1.0060x over previous
"""Trainium2 Bass kernel for nn_COMSAGPool_gumble (GNN message passing).

Math (reference):
    prop(x) = nd * segsum_dst((ns * x)[src]),  ns/nd = rsqrt(clipped degrees)
    out1 = prop(F) @ W1 + b1 ; out2 = prop(out1 @ W2) + b2
    mask = onehot(argmax(out2 + gumbel)); p = softmax(out2)
    fdis = F * tanh(p0) * mask0 ; fcom = F * tanh(p1) * mask1

Rewrite (prop is linear, commutes with right-mult):
    out2 = prop(prop(F @ (W1@W2))) + q * (b1@W2) + b2,  q = prop(ones)
so both sparse hops run on 2 feature columns.

Distribution: nodes sharded over 8 cores; node->slot assignment is
degree-sorted so each slot column has a uniform per-node edge capacity
(fixed-stride segment sums, no boundary extraction). Per-edge values are
fetched with dma_gather (256B items, 4 node-values packed per item,
sub-position resolved with host-built masks). Hop outputs are exchanged
with AllGather; the host reassembles outputs from slot order.
"""
import os
import numpy as np

import concourse.bacc as bacc
import concourse.mybir as mybir
import concourse.tile as tile
from concourse import bass_utils, library_config
from concourse.tile_rust import add_dep_helper

N = 100000
E = 1600000
D = 128
NCORES = 8
TC = 98                      # slot columns; 8*128*98 = 100352 slots
SLOTS = 128 * TC             # 12544 per core
GRID = NCORES * 128          # 1024 grid rows (core, partition)
PADQ = 2
NITEMS = (NCORES * SLOTS) // 4   # 25088 packed 256B items
EW = 64                      # floats per item
NIDX_CALL = 1024             # dma_gather per-call cap
CCB = 8                      # calls per staging buffer (8*1024/128 = 64 cols)
NQ = 4
F32 = mybir.dt.float32

_CACHE = {}
LAST_RESULTS = None


def _prep(src, dst, feature, gumbel, W1, b1, W2, b2):
    src = np.asarray(src).astype(np.int64)
    dst = np.asarray(dst).astype(np.int64)

    deg_out = np.bincount(src, minlength=N)
    deg_in = np.bincount(dst, minlength=N)
    ns = (1.0 / np.sqrt(np.maximum(deg_out, 1).astype(np.float32))).astype(np.float32)
    nd = (1.0 / np.sqrt(np.maximum(deg_in, 1).astype(np.float32))).astype(np.float32)

    # degree-sorted node -> (core, partition, column) assignment
    degK = ((deg_in + PADQ - 1) // PADQ) * PADQ
    order_nodes = np.argsort(-degK, kind="stable")
    rank = np.empty(N, np.int64)
    rank[order_nodes] = np.arange(N)
    t_n = rank // GRID
    g_n = rank % GRID
    c_n = g_n // 128
    p_n = g_n % 128
    # per-column edge capacity (uniform across the whole grid)
    ncols_used = (N + GRID - 1) // GRID
    Kt = np.zeros(TC, np.int64)
    first = np.minimum(np.arange(ncols_used) * GRID, N - 1)
    Kt[:ncols_used] = degK[order_nodes[first]]
    off_t = np.zeros(TC + 1, np.int64)
    np.cumsum(Kt, out=off_t[1:])
    L = int(off_t[-1])
    Lpad = ((L + CCB * 8 - 1) // (CCB * 8)) * (CCB * 8)
    ncalls = (Lpad * 128) // NIDX_CALL

    # cc row u, packed item, sub-position
    u_n = c_n * SLOTS + p_n * TC + t_n
    item_n = (u_n // 4).astype(np.int16)
    sub_n = (u_n % 4).astype(np.int8)

    # edge -> (core, partition, column, slot col j)
    cd, pd, td = c_n[dst], p_n[dst], t_n[dst]
    gkey = ((cd * 128 + pd) * TC + td).astype(np.int64)
    eorder = np.argsort(gkey, kind="stable")
    gkey_s = gkey[eorder]
    cnt = np.bincount(gkey, minlength=GRID * TC)
    gstart = np.zeros(GRID * TC + 1, np.int64)
    np.cumsum(cnt, out=gstart[1:])
    erank = np.arange(E, dtype=np.int64) - gstart[gkey_s]
    j_s = off_t[gkey_s % TC] + erank
    assert int(j_s.max()) < L

    # per-core gather item grids + sub-position masks
    items = np.zeros((NCORES, 128, Lpad), np.int16)
    masks = np.zeros((NCORES, 4, 128, Lpad), np.float32)
    cds, pds = cd[eorder], pd[eorder]
    src_s = src[eorder]
    items[cds, pds, j_s] = item_n[src_s]
    m_sel = sub_n[src_s]
    for k in range(4):
        sel = m_sel == k
        masks[cds[sel], k, pds[sel], j_s[sel]] = 1.0

    # wrapped int16 idx tiles: call q covers seq i = (col-j local)*128 + p
    seq = items.transpose(0, 2, 1).reshape(NCORES, Lpad * 128)
    seq = seq.reshape(NCORES, ncalls, NIDX_CALL // 16, 16)
    idxw = np.zeros((NCORES, 128, ncalls * (NIDX_CALL // 16)), np.int16)
    for p in range(128):
        idxw[:, p, :] = seq[:, :, :, p % 16].reshape(NCORES, -1)

    # contiguous column ranges with equal K > 0
    ranges = []
    t0 = 0
    while t0 < TC:
        if Kt[t0] == 0:
            t0 += 1
            continue
        t1 = t0
        while t1 < TC and Kt[t1] == Kt[t0]:
            t1 += 1
        ranges.append((int(t0), int(t1), int(Kt[t0]), int(off_t[t0])))
        t0 = t1

    def plane(val):
        a = np.zeros((NCORES, 128, TC), np.float32)
        a[c_n, p_n, t_n] = val
        return a

    def plane2(val):
        a = plane(val)
        return np.repeat(a.reshape(NCORES, 128, TC, 1), 2, axis=3).reshape(
            NCORES, 128, 2 * TC
        )

    W1 = np.asarray(W1, np.float32)
    W2 = np.asarray(W2, np.float32)
    b1 = np.asarray(b1, np.float32)
    b2 = np.asarray(b2, np.float32)
    W12 = (W1 @ W2).astype(np.float32)
    b1W2 = (b1 @ W2).astype(np.float32)
    q_vec = (np.bincount(dst, weights=ns[src].astype(np.float64), minlength=N)
             .astype(np.float32) * nd)
    qb2 = np.zeros((NCORES, 128, 2 * TC), np.float32)
    qp = plane(q_vec)
    for ch in range(2):
        qb2[:, :, ch::2] = qp * b1W2[ch] + b2[ch]
    gum = np.asarray(gumbel, np.float32)
    gd = plane(gum[:, 0] - gum[:, 1])

    slot2node = np.full((NCORES, SLOTS), -1, np.int64)
    slot2node[c_n, p_n * TC + t_n] = np.arange(N)
    feature = np.asarray(feature, np.float32)
    fs, fts = [], []
    for c in range(NCORES):
        s2n = slot2node[c]
        fc = np.zeros((SLOTS, D), np.float32)
        valid = s2n >= 0
        fc[valid] = feature[s2n[valid]]
        fs.append(fc)
        # FT column t*128+p must hold slot p*TC+t (the P' matmul writes
        # column-tile t to psum partitions p)
        j = np.arange(SLOTS)
        perm = (j % 128) * TC + (j // 128)
        fts.append(np.ascontiguousarray(fc[perm].T))

    return dict(
        L=L, Lpad=Lpad, ncalls=ncalls, ranges=ranges,
        idxw=idxw, masks=masks, fs=fs, fts=fts, W12=W12,
        ns2=plane2(ns), v1=plane2(nd * ns), nd2=plane2(nd), qb2=qb2, gd=gd,
        slot2node=slot2node,
    )


def _build(Lpad, ncalls, ranges, debug=False):
    nc = bacc.Bacc("TRN2", target_bir_lowering=False, debug=False,
                   num_devices=NCORES, num_swdge_queues=NQ)
    AOT = mybir.AluOpType
    groups = [list(range(NCORES))]

    def di(name, shape, dt=F32):
        return nc.dram_tensor(name, shape, dt, kind="ExternalInput").ap()

    FT = di("ft", [128, SLOTS])
    W12 = di("w12", [128, 2])
    IDX = di("idx", [128, ncalls * (NIDX_CALL // 16)], mybir.dt.int16)
    MK = di("mk", [4, 128, Lpad])
    NS2 = di("ns2", [128, 2 * TC])
    V1 = di("v1", [128, 2 * TC])
    ND2 = di("nd2", [128, 2 * TC])
    QB2 = di("qb2", [128, 2 * TC])
    GD = di("gd", [128, TC])
    F = di("f", [SLOTS, D])

    FDIS = nc.dram_tensor("fdis", [SLOTS, D], F32, kind="ExternalOutput").ap()
    FCOM = nc.dram_tensor("fcom", [SLOTS, D], F32, kind="ExternalOutput").ap()
    M0 = nc.dram_tensor("m0", [SLOTS], F32, kind="ExternalOutput").ap()
    M1 = nc.dram_tensor("m1", [SLOTS], F32, kind="ExternalOutput").ap()

    ccPin = nc.dram_tensor("ccpin", [SLOTS, 2], F32, kind="Internal").ap()
    ccPout = nc.dram_tensor("ccpout", [NCORES * SLOTS, 2], F32,
                            kind="Internal", addr_space="Shared").ap()
    ccQin = nc.dram_tensor("ccqin", [SLOTS, 2], F32, kind="Internal").ap()
    ccQout = nc.dram_tensor("ccqout", [NCORES * SLOTS, 2], F32,
                            kind="Internal", addr_space="Shared").ap()
    padP = nc.dram_tensor("padp", [NITEMS, EW], F32, kind="Internal").ap()
    padQ = nc.dram_tensor("padq", [NITEMS, EW], F32, kind="Internal").ap()
    barin = nc.dram_tensor("barin", [128, 2], F32, kind="Internal").ap()
    barout = nc.dram_tensor("barout", [128, 2], F32, kind="Internal").ap()
    dbg = {}
    if debug:
        for nm, shp in [("dbg_v0", [128, Lpad]), ("dbg_v1", [128, Lpad]),
                        ("dbg_cmp", [128, NCORES * SLOTS * 2 // 128]),
                        ("dbg_s0", [128, TC]), ("dbg_s1", [128, TC])]:
            dbg[nm] = nc.dram_tensor(nm, shp, F32, kind="ExternalOutput").ap()

    with tile.TileContext(nc) as tc:
        with (
            tc.tile_pool(name="const", bufs=1) as kpool,
            tc.tile_pool(name="gbuf", bufs=2) as gpool,
            tc.tile_pool(name="vals", bufs=1) as vpool,
            tc.tile_pool(name="nodes", bufs=1) as npool,
            tc.tile_pool(name="fio", bufs=2) as fpool,
            tc.tile_pool(name="fpre", bufs=4) as prepool,
            tc.tile_pool(name="psum", bufs=1, space="PSUM") as ppool,
        ):
            nc.gpsimd.load_library(library_config.mlp)
            idx_sb = kpool.tile([128, ncalls * (NIDX_CALL // 16)], mybir.dt.int16)
            nc.sync.dma_start(idx_sb[:], IDX)
            mk_sb = []
            for k in range(4):
                t = kpool.tile([128, Lpad], F32, tag=f"mk{k}")
                nc.sync.dma_start(t[:], MK[k])
                mk_sb.append(t)
            w12_sb = kpool.tile([128, 2], F32)
            nc.sync.dma_start(w12_sb[:], W12)
            ns2_sb = kpool.tile([128, 2 * TC], F32)
            nc.sync.dma_start(ns2_sb[:], NS2)
            v1_sb = kpool.tile([128, 2 * TC], F32)
            nc.sync.dma_start(v1_sb[:], V1)
            nd2_sb = kpool.tile([128, 2 * TC], F32)
            nc.sync.dma_start(nd2_sb[:], ND2)
            qb2_sb = kpool.tile([128, 2 * TC], F32)
            nc.sync.dma_start(qb2_sb[:], QB2)
            gd_sb = kpool.tile([128, TC], F32)
            nc.sync.dma_start(gd_sb[:], GD)

            # ---- P' = ns * (F @ W12) via PE (FT is host-transposed F) ----
            with tc.tile_pool(name="ftw", bufs=2) as ftpool:
                p_psum = ppool.tile([128, 2 * TC], F32)
                FTCH = 8
                for t0c in range(0, TC, FTCH):
                    nbt = min(FTCH, TC - t0c)
                    ft_sb = ftpool.tile([128, FTCH * 128], F32, tag="ftc")
                    nc.sync.dma_start(
                        ft_sb[:, 0 : nbt * 128],
                        FT[:, t0c * 128 : (t0c + nbt) * 128],
                    )
                    for j in range(nbt):
                        t = t0c + j
                        nc.tensor.matmul(
                            p_psum[:, 2 * t : 2 * t + 2],
                            lhsT=ft_sb[:, j * 128 : (j + 1) * 128],
                            rhs=w12_sb[:],
                            start=True, stop=True,
                        )
                pp_sb = npool.tile([128, 2 * TC], F32, tag="pp")
                nc.vector.tensor_tensor(pp_sb[:], p_psum[:], ns2_sb[:], AOT.mult)

            def exchange(plane_sb, cc_in, cc_out, pad):
                """compact write -> AllGather -> expand into packed items."""
                nc.sync.dma_start(
                    cc_in.rearrange("(p t) c -> p (t c)", p=128), plane_sb[:]
                )
                nc.gpsimd.collective_compute(
                    "AllGather", AOT.bypass, replica_groups=groups,
                    ins=[cc_in], outs=[cc_out],
                )
                # barrier collective: ring-ordered execution means its
                # completion implies the AllGather's remote data has landed
                bar = nc.gpsimd.collective_compute(
                    "AllReduce", AOT.add, replica_groups=groups,
                    ins=[barin], outs=[barout],
                )
                cmp_sb = vpool.tile([128, NCORES * SLOTS * 2 // 128], F32,
                                    tag="cmp")
                rd = nc.sync.dma_start(
                    cmp_sb[:], cc_out.rearrange("(p a) c -> p (a c)", p=128)
                )
                add_dep_helper(rd.ins, bar.ins, reason="cc data barrier")
                if debug and cc_in is ccPin:
                    nc.sync.dma_start(dbg["dbg_cmp"], cmp_sb[:])
                nc.sync.dma_start(
                    pad.rearrange("(p i) w -> p i w", p=128)[:, :, 0:8],
                    cmp_sb[:].rearrange("p (i e) -> p i e", e=8),
                )

            exchange(pp_sb, ccPin, ccPout, padP)

            # ---- one hop: gather packed items, mask-select, fixed-K reduce --
            def hop(pad_src, tag):
                val0 = vpool.tile([128, Lpad], F32, tag="va")
                val1 = vpool.tile([128, Lpad], F32, tag="vb")
                bcols = CCB * NIDX_CALL // 128      # columns per staging buf
                nblk = Lpad // bcols
                for b in range(nblk):
                    gt = gpool.tile([128, bcols, EW], F32, tag="g")
                    for qq in range(CCB):
                        q = b * CCB + qq
                        nc.gpsimd.dma_gather(
                            out_ap=gt[:, qq * 8 : (qq + 1) * 8, :],
                            in_ap=pad_src,
                            idxs_ap=idx_sb[
                                :,
                                q * (NIDX_CALL // 16) : (q + 1) * (NIDX_CALL // 16),
                            ],
                            num_idxs=NIDX_CALL,
                            num_idxs_reg=NIDX_CALL,
                            elem_size=EW,
                            queue_num=q % NQ,
                        )
                    c0 = b * bcols
                    for cch, vt in ((0, val0), (1, val1)):
                        tmp = gpool.tile([128, bcols], F32, tag=f"t{cch}")
                        tmp2 = gpool.tile([128, bcols], F32, tag=f"u{cch}")
                        nc.vector.tensor_tensor(
                            tmp[:], gt[:, :, 0 + cch],
                            mk_sb[0][:, c0 : c0 + bcols], AOT.mult)
                        nc.vector.tensor_tensor(
                            tmp2[:], gt[:, :, 2 + cch],
                            mk_sb[1][:, c0 : c0 + bcols], AOT.mult)
                        nc.vector.tensor_tensor(tmp[:], tmp[:], tmp2[:], AOT.add)
                        nc.vector.tensor_tensor(
                            tmp2[:], gt[:, :, 4 + cch],
                            mk_sb[2][:, c0 : c0 + bcols], AOT.mult)
                        nc.vector.tensor_tensor(tmp[:], tmp[:], tmp2[:], AOT.add)
                        nc.vector.tensor_tensor(
                            tmp2[:], gt[:, :, 6 + cch],
                            mk_sb[3][:, c0 : c0 + bcols], AOT.mult)
                        nc.vector.tensor_tensor(
                            vt[:, c0 : c0 + bcols], tmp[:], tmp2[:], AOT.add)
                if debug and tag == "a":
                    nc.sync.dma_start(dbg["dbg_v0"], val0[:])
                    nc.sync.dma_start(dbg["dbg_v1"], val1[:])
                seg0 = npool.tile([128, TC], F32, tag=f"sa{tag}")
                seg1 = npool.tile([128, TC], F32, tag=f"sb{tag}")
                nc.vector.memset(seg0[:], 0.0)
                nc.vector.memset(seg1[:], 0.0)
                for (t0, t1, K, off) in ranges:
                    ntr = t1 - t0
                    for vt, st in ((val0, seg0), (val1, seg1)):
                        nc.vector.tensor_reduce(
                            st[:, t0:t1],
                            vt[:, off : off + ntr * K].rearrange(
                                "p (a k) -> p a k", k=K
                            ),
                            axis=mybir.AxisListType.X,
                            op=AOT.add,
                        )
                return seg0, seg1

            s10, s11 = hop(padP, "a")
            if debug:
                nc.sync.dma_start(dbg["dbg_s0"], s10[:])
                nc.sync.dma_start(dbg["dbg_s1"], s11[:])
            qp_sb = npool.tile([128, 2 * TC], F32, tag="qp")
            qp3 = qp_sb[:].rearrange("p (t c) -> p t c", c=2)
            v13 = v1_sb[:].rearrange("p (t c) -> p t c", c=2)
            nc.vector.tensor_tensor(qp3[:, :, 0], s10[:], v13[:, :, 0], AOT.mult)
            nc.vector.tensor_tensor(qp3[:, :, 1], s11[:], v13[:, :, 1], AOT.mult)
            exchange(qp_sb, ccQin, ccQout, padQ)
            s20, s21 = hop(padQ, "b")

            # ---- head ----
            nd3 = nd2_sb[:].rearrange("p (t c) -> p t c", c=2)
            qb3 = qb2_sb[:].rearrange("p (t c) -> p t c", c=2)
            o20a = npool.tile([128, TC], F32, tag="o20a")
            o21a = npool.tile([128, TC], F32, tag="o21a")
            nc.vector.tensor_tensor(o20a[:], s20[:], nd3[:, :, 0], AOT.mult)
            nc.vector.tensor_tensor(o21a[:], s21[:], nd3[:, :, 1], AOT.mult)
            o20 = npool.tile([128, TC], F32, tag="o20")
            o21 = npool.tile([128, TC], F32, tag="o21")
            nc.vector.tensor_tensor(o20[:], o20a[:], qb3[:, :, 0], AOT.add)
            nc.vector.tensor_tensor(o21[:], o21a[:], qb3[:, :, 1], AOT.add)
            d01 = npool.tile([128, TC], F32, tag="d01")
            nc.vector.tensor_tensor(d01[:], o20[:], o21[:], AOT.subtract)
            m = npool.tile([128, TC], F32, tag="m")
            nc.vector.tensor_tensor(m[:], d01[:], gd_sb[:], AOT.add)
            mask0 = npool.tile([128, TC], F32, tag="mask0")
            mask1 = npool.tile([128, TC], F32, tag="mask1")
            nc.vector.tensor_scalar(mask0[:], m[:], 0.0, None, AOT.is_ge)
            nc.vector.tensor_scalar(mask1[:], m[:], 0.0, None, AOT.is_lt)
            p0 = npool.tile([128, TC], F32, tag="p0")
            p1 = npool.tile([128, TC], F32, tag="p1")
            nc.scalar.activation(p0[:], d01[:], mybir.ActivationFunctionType.Sigmoid)
            nc.scalar.activation(p1[:], d01[:], mybir.ActivationFunctionType.Sigmoid,
                                 scale=-1.0)
            t0_ = npool.tile([128, TC], F32, tag="t0")
            t1_ = npool.tile([128, TC], F32, tag="t1")
            nc.scalar.activation(t0_[:], p0[:], mybir.ActivationFunctionType.Tanh)
            nc.scalar.activation(t1_[:], p1[:], mybir.ActivationFunctionType.Tanh)
            s0 = npool.tile([128, TC], F32, tag="s0")
            s1 = npool.tile([128, TC], F32, tag="s1")
            nc.vector.tensor_tensor(s0[:], t0_[:], mask0[:], AOT.mult)
            nc.vector.tensor_tensor(s1[:], t1_[:], mask1[:], AOT.mult)

            nc.sync.dma_start(M0.rearrange("(p t) -> p t", p=128), mask0[:])
            nc.sync.dma_start(M1.rearrange("(p t) -> p t", p=128), mask1[:])

            # ---- final: fdis/fcom = F * s0/s1, batched over t ----
            NB = 4
            for tb in range(0, TC, NB):
                nb = min(NB, TC - tb)
                fin = prepool.tile([128, NB, D], F32, tag="fin")
                nc.sync.dma_start(
                    fin[:, 0:nb, :],
                    F.rearrange("(p t) d -> p t d", p=128)[:, tb : tb + nb, :],
                )
                fd = fpool.tile([128, NB, D], F32, tag="fd")
                fc = fpool.tile([128, NB, D], F32, tag="fc")
                for j in range(nb):
                    t = tb + j
                    nc.vector.tensor_scalar(
                        fd[:, j, :], fin[:, j, :], s0[:, t : t + 1], None, AOT.mult)
                    nc.vector.tensor_scalar(
                        fc[:, j, :], fin[:, j, :], s1[:, t : t + 1], None, AOT.mult)
                nc.sync.dma_start(
                    FDIS.rearrange("(p t) d -> p t d", p=128)[:, tb : tb + nb, :],
                    fd[:, 0:nb, :])
                nc.sync.dma_start(
                    FCOM.rearrange("(p t) d -> p t d", p=128)[:, tb : tb + nb, :],
                    fc[:, 0:nb, :])

    nc.compile()
    return nc


def kernel(feature, src, dst, gumbel, label, W1, b1, W2, b2):
    global LAST_RESULTS
    pre = _prep(src, dst, feature, gumbel, W1, b1, W2, b2)
    key = (pre["Lpad"], pre["ncalls"], tuple(pre["ranges"]))
    debug = bool(os.environ.get("KERNEL_DEBUG"))
    key = key + (debug,)
    if key not in _CACHE:
        _CACHE[key] = _build(pre["Lpad"], pre["ncalls"], pre["ranges"],
                             debug=debug)
    nc = _CACHE[key]

    in_maps = []
    for c in range(NCORES):
        in_maps.append({
            "ft": pre["fts"][c],
            "w12": np.ascontiguousarray(pre["W12"]),
            "idx": np.ascontiguousarray(pre["idxw"][c]),
            "mk": np.ascontiguousarray(pre["masks"][c]),
            "ns2": np.ascontiguousarray(pre["ns2"][c]),
            "v1": np.ascontiguousarray(pre["v1"][c]),
            "nd2": np.ascontiguousarray(pre["nd2"][c]),
            "qb2": np.ascontiguousarray(pre["qb2"][c]),
            "gd": np.ascontiguousarray(pre["gd"][c]),
            "f": pre["fs"][c],
        })

    trace = os.environ.get("KERNEL_TRACE", "") not in ("", "0")
    try:
        res = bass_utils.run_bass_kernel_spmd(
            nc, in_maps, core_ids=list(range(NCORES)), trace=trace
        )
    except ModuleNotFoundError:
        os.environ["BASS_NEVER_TRACE"] = "1"
        res = bass_utils.run_bass_kernel_spmd(
            nc, in_maps, core_ids=list(range(NCORES))
        )
    LAST_RESULTS = res

    fdis = np.zeros((N, D), np.float32)
    fcom = np.zeros((N, D), np.float32)
    m0 = np.zeros(N, np.float32)
    m1 = np.zeros(N, np.float32)
    for c in range(NCORES):
        s2n = pre["slot2node"][c]
        valid = s2n >= 0
        nodes = s2n[valid]
        fdis[nodes] = res.results[c]["fdis"][valid]
        fcom[nodes] = res.results[c]["fcom"][valid]
        m0[nodes] = res.results[c]["m0"][valid]
        m1[nodes] = res.results[c]["m1"][valid]
    return fdis, fcom, m0, m1
